# revision 1
# baseline (speedup 1.0000x reference)
"""DCNv2 on 8 trn2 cores, data-parallel over batch. v2: bf16 DVE-2x MAC with
i-inner layout + per-(band,kx) dynamic skipping of the 12 outer tent terms.

Per core (one image):
  phase1: offset/mask convs (z-chunk + x-chunk PSUM accumulation), OM bf16
  phase2: PE-transpose OM -> OMT[j, 27, i] bf16
  per band (R=8 rows):
    flags: |d|>0.99 mask -> ones-matmul -> reduce -> FLS[1,6] (y/x per kx)
    projection: image-row-stationary matmuls -> VT2 (|ex|<=1 pairs, bufs=2)
                and VT3 (|ex|=2 pairs, bufs=1 + dummy-touch for WAR)
      slot-contiguous storage: [j, pair, ky, o, s(16)] so MAC runs are i-inner
    tents: TY/TX (scalar) -> TYM=TY*MSK (DVE) -> CT2=(TYM*2)*TX (DVE STT)
    MAC: bf16 tensor_tensor mult+add pairs, dims (ky,o,i) inner i step1:
      gpsimd: (kx, ey=0, ex=0) -> ACCB; DVE inner 8/kx -> ACCA
      tc.If(flagy>0): ey=+-2 terms; tc.If(flagx>0): ex=+-2 terms (DVE only)
    collapse ACCA+ACCB, ky-sum, f32 convert, DMA out
"""
import sys

sys.path.insert(0, "/opt/trn_rl_repo")

import numpy as np

import concourse.bass as bass
import concourse.mybir as mybir
import concourse.tile as tile
from concourse.bass_utils import run_bass_kernel_spmd

F32 = mybir.dt.float32
BF16 = mybir.dt.bfloat16
ALU = mybir.AluOpType
ACTF = mybir.ActivationFunctionType

H = W = 128
C = O = 64
KK = 9
PW = 134          # padded width/height, image at [3, 131)
R = 8             # output rows per band
NBANDS = H // R
SLOTS = 14        # source rows per band: padded rows [i0, i0+14)
SLEN = 16         # slot-dim storage (s0=1 origin => ey=+-1 terms 4B-aligned)
S0 = 1
NCORES = 8
DYN_SKIP = False

# (sx, kx) pairs, sx = kx - 1 + ex.  main: |ex|<=1, outer: |ex|=2
MAIN_SX = {}      # sx -> (kxlo, kxhi)
for sx in range(-2, 3):
    kxs = [kx for kx in range(3) if abs(sx - kx + 1) <= 1]
    MAIN_SX[sx] = (min(kxs), max(kxs))
MAIN_PAIRS = []   # ordered (sx, kx)
for sx in range(-2, 3):
    lo, hi = MAIN_SX[sx]
    for kx in range(lo, hi + 1):
        MAIN_PAIRS.append((sx, kx))
MAIN_IDX = {p: i for i, p in enumerate(MAIN_PAIRS)}        # 9 pairs
OUT_PAIRS = [(kx - 1 + ex, kx) for kx in range(3) for ex in (-2, 2)]
OUT_IDX = {p: i for i, p in enumerate(OUT_PAIRS)}          # 6 pairs
PBLK = 3 * O * SLEN   # 3072 elems per (sx,kx) pair block
KYSTR = O * SLEN      # 1024
SLEN3 = 12            # VT3 slot dim: stores s-2 for s in [2,14)
PBLK3 = 3 * O * SLEN3
KYSTR3 = O * SLEN3

INNER = [(ey, ex) for ey in (-1, 0, 1) for ex in (-1, 0, 1)]
OUTER_Y = [(ey, ex) for ey in (-2, 2) for ex in (-1, 0, 1)]
OUTER_X = [(ey, ex) for ey in (-1, 0, 1) for ex in (-2, 2)]


def _fix_multiwait(nc, max_waits=1):
    import bass_rust

    ctr = 0
    for f in nc.m.functions:
        for bb in f.blocks:
            insts = bb.instructions

            def nwaits(i):
                si = i.sync_info
                return len(si.on_wait) if si is not None else 0

            if not any(nwaits(i) > max_waits for i in insts):
                continue
            out = []
            for inst in insts:
                si = inst.sync_info
                waits = list(si.on_wait) if si is not None else []
                if len(waits) > max_waits:
                    extra, keep = waits[:-max_waits], waits[-max_waits:]
                    for j in range(0, len(extra), max_waits):
                        ctr += 1
                        nop = mybir.InstNoOp(name=f"WFIX-{ctr}", ins=[], outs=[])
                        nop.engine = inst.engine
                        nop.sync_info = bass_rust.SyncInfo(
                            on_wait=extra[j : j + max_waits], on_update=[]
                        )
                        out.append(nop)
                    inst.sync_info = bass_rust.SyncInfo(
                        on_wait=keep, on_update=list(si.on_update)
                    )
                out.append(inst)
            bb.instructions = out


def _dbg_regs(nc, label):
    import os
    if not os.environ.get("V2_DBG_REGS"):
        return
    regs = []
    try:
        while True:
            regs.append(nc.alloc_register(mybir.EngineType.DVE))
    except Exception:
        pass
    for r in regs:
        nc.free_register(r)
    print(f"[regs] {label}: {len(regs)} free", flush=True)


def build_nc(fix_waits=True, dyn_skip=DYN_SKIP):
    nc = bass.Bass()
    zin = nc.dram_tensor("zin", [64, PW * PW], BF16, kind="ExternalInput")
    xin = nc.dram_tensor("xin", [64, PW * PW], BF16, kind="ExternalInput")
    wcz = nc.dram_tensor("wcz", [64, KK * 27], BF16, kind="ExternalInput")
    wcx = nc.dram_tensor("wcx", [64, KK * 27], BF16, kind="ExternalInput")
    wflat = nc.dram_tensor("wflat", [C, 3 * 3 * O], BF16, kind="ExternalInput")
    bias27 = nc.dram_tensor("bias27", [27, 1], F32, kind="ExternalInput")
    identb = nc.dram_tensor("identb", [32, 32], BF16, kind="ExternalInput")
    cstb = nc.dram_tensor("cstb", [128, 8], BF16, kind="ExternalInput")
    onesb = nc.dram_tensor("onesb", [128, 1], BF16, kind="ExternalInput")
    outD = nc.dram_tensor("outD", [128, NBANDS, O * R], BF16, kind="ExternalOutput")
    flsD = (nc.dram_tensor("flsD", [1, NBANDS, 24], F32, kind="ExternalOutput")
            if dyn_skip else None)

    def rap(t, off, dims):
        a = t[:]
        return bass.AP(tensor=a.tensor, offset=a.offset + off, ap=dims)

    with tile.TileContext(nc) as tc:
        with tc.tile_pool(name="persist", bufs=1) as pp:
            X = pp.tile([64, PW, PW], BF16)
            WF = pp.tile([64, 3 * 3 * O], BF16)   # [c, kx*192 + ky*64 + o]
            OMT = pp.tile([128, 27, H], BF16)     # [j, plane, i]
            CSTB = pp.tile([128, 8], BF16)        # [-2,-1,0,1,2, 1.0, 0, 0]
            ONES = pp.tile([128, 1], BF16)
            BIA = pp.tile([27, 1], F32)
            IDTB = pp.tile([32, 32], BF16)
            nc.sync.dma_start(X[:], xin.rearrange("p (a b) -> p a b", b=PW))
            nc.sync.dma_start(WF[:], wflat[:])
            nc.sync.dma_start(CSTB[:], cstb[:])
            nc.sync.dma_start(ONES[:], onesb[:])
            nc.sync.dma_start(BIA[:], bias27[:])
            nc.sync.dma_start(IDTB[:], identb[:])

            # ---- phase 1+2: offset/mask convs, then transpose to OMT
            with (
                tc.tile_pool(name="ph1", bufs=1) as p1,
                tc.tile_pool(name="psc", bufs=2, space="PSUM") as pconv,
                tc.tile_pool(name="pst", bufs=2, space="PSUM") as ptr,
            ):
                Z = p1.tile([64, PW, PW], BF16)
                WCZ = p1.tile([64, KK, 27], BF16)
                WCX = p1.tile([64, KK, 27], BF16)
                OM = p1.tile([27, H, W], BF16)
                nc.sync.dma_start(Z[:], zin.rearrange("p (a b) -> p a b", b=PW))
                nc.sync.dma_start(WCZ[:], wcz.rearrange("p (t q) -> p t q", q=27))
                nc.sync.dma_start(WCX[:], wcx.rearrange("p (t q) -> p t q", q=27))
                for nt in range(32):  # 4 image rows per PSUM tile
                    r0 = nt * 4
                    ps = pconv.tile([27, 512], F32, tag="convps")
                    for t in range(KK):
                        ty, tx = t // 3, t % 3
                        zr = Z[:, r0 + 2 + ty : r0 + 6 + ty, 2 + tx : 2 + tx + W]
                        xr = X[:, r0 + 2 + ty : r0 + 6 + ty, 2 + tx : 2 + tx + W]
                        nc.tensor.matmul(
                            ps[:], WCZ[:, t, :], zr, start=(t == 0), stop=False
                        )
                        nc.tensor.matmul(
                            ps[:], WCX[:, t, :], xr, start=False, stop=(t == KK - 1)
                        )
                    ps3 = ps[:].rearrange("p (a b) -> p a b", b=W)
                    nc.scalar.activation(
                        OM[:, r0 : r0 + 4, :], ps3, ACTF.Identity, bias=BIA[:, 0:1]
                    )
                for i in range(H):
                    pt = ptr.tile([128, 32], BF16, tag="trps")
                    nc.tensor.transpose(pt[:, 0:27], OM[:, i, :], IDTB[0:27, 0:27])
                    nc.scalar.copy(OMT[:, :, i], pt[:, 0:27])

            _dbg_regs(nc, "after phase1/2")
            # ---- phase 3: per-band
            with (
                tc.tile_pool(name="vt2p", bufs=2) as pvt2,
                tc.tile_pool(name="vt3p", bufs=1) as pvt3,
                tc.tile_pool(name="band", bufs=2) as pb,
                tc.tile_pool(name="bscr", bufs=1) as pb1,
                tc.tile_pool(name="accs", bufs=1) as pacc,
                tc.tile_pool(name="psv", bufs=4, space="PSUM") as pproj,
                tc.tile_pool(name="psf", bufs=1, space="PSUM") as pfl,
            ):
                import os
                no_flags = (not dyn_skip) or bool(os.environ.get("V2_NO_FLAGS"))
                for ib in range(NBANDS):
                    i0 = ib * R

                    # -- flags: any |dy|>0.99 / |dx|>0.99 per kx
                    if not no_flags:
                        AB = pb1.tile([128, 18, R], BF16, tag="ab")
                        nc.scalar.activation(
                            AB[:], OMT[:, 0:18, i0 : i0 + R], ACTF.Abs
                        )
                        MV = pb1.tile([128, 18, R], BF16, tag="mv")
                        nc.scalar.activation(
                            MV[:], AB[:], ACTF.Relu, bias=CSTB[:, 6:7]
                        )
                        psF = pfl.tile([1, 144], F32, tag="flps")
                        nc.tensor.matmul(
                            psF[:], ONES[:, 0:1],
                            MV[:].rearrange("p a b -> p (a b)"),
                            start=True, stop=True,
                        )
                        FLP = pb1.tile([1, 18], F32, tag="flp")
                        nc.vector.tensor_reduce(
                            out=FLP[:],
                            in_=psF[:].rearrange("p (a b) -> p a b", b=8),
                            axis=mybir.AxisListType.X, op=ALU.add,
                        )
                        FLS = pb1.tile([1, 2, 3], F32, tag="fls")
                        for ax in range(2):
                            nc.vector.tensor_reduce(
                                out=FLS[0:1, ax, :],
                                in_=rap(FLP, 9 * ax, [[18, 1], [1, 3], [3, 3]]),
                                axis=mybir.AxisListType.X, op=ALU.add,
                            )
                        nc.sync.dma_start(flsD[0:1, ib, 0:18], FLP[:])
                        nc.sync.dma_start(flsD[0:1, ib, 18:24],
                                          FLS[:].rearrange("p a b -> p (a b)"))

                    # -- projections into VT2 (main) / VT3 (outer)
                    VT2 = pvt2.tile([128, 9 * PBLK], BF16, tag="vt2")
                    VT3 = pvt3.tile([128, 6 * PBLK3], BF16, tag="vt3")
                    for s in range(S0, S0 + SLOTS):
                        prow = i0 + s - S0
                        for sx in range(-2, 3):
                            kxlo, kxhi = MAIN_SX[sx]
                            nkx = kxhi - kxlo + 1
                            ncols = nkx * 3 * O
                            ps = pproj.tile([128, 576], F32, tag="vtps")
                            lhsT = X[:, prow, 3 + sx : 3 + sx + W]
                            rhs = WF[:, kxlo * 192 : kxlo * 192 + ncols]
                            if ncols <= 512:
                                nc.tensor.matmul(
                                    ps[:, 0:ncols], lhsT, rhs, start=True, stop=True
                                )
                            else:
                                nc.tensor.matmul(
                                    ps[:, 0:512], lhsT, rhs[:, 0:512],
                                    start=True, stop=True,
                                )
                                nc.tensor.matmul(
                                    ps[:, 512:ncols], lhsT, rhs[:, 512:ncols],
                                    start=True, stop=True,
                                )
                            p0 = MAIN_IDX[(sx, kxlo)]
                            nc.scalar.copy(
                                rap(VT2, p0 * PBLK + s,
                                    [[9 * PBLK, 128], [KYSTR, 3 * nkx], [SLEN, O]]),
                                rap(ps, 0, [[576, 128], [O, 3 * nkx], [1, O]]),
                            )
                        # outer pairs: per kx the two ex=+-2 shifts
                        # (VT3 only serves s in [2,14): stored at s-2)
                        if 2 <= s < 14:
                            for pi, (sx, kx) in enumerate(OUT_PAIRS):
                                ps = pproj.tile([128, 576], F32, tag="vtps")
                                lhsT = X[:, prow, 3 + sx : 3 + sx + W]
                                rhs = WF[:, kx * 192 : kx * 192 + 192]
                                nc.tensor.matmul(
                                    ps[:, 0:192], lhsT, rhs, start=True, stop=True
                                )
                                nc.vector.tensor_copy(
                                    rap(VT3, pi * PBLK3 + (s - 2),
                                        [[6 * PBLK3, 128], [KYSTR3, 3],
                                         [SLEN3, O]]),
                                    rap(ps, 0, [[576, 128], [O, 3], [1, O]]),
                                )

                    _dbg_regs(nc, f"band{ib} after proj")
                    # -- tents
                    MSK = pb.tile([128, 9, R], BF16, tag="msk")
                    nc.scalar.activation(
                        MSK[:], OMT[:, 18:27, i0 : i0 + R], ACTF.Sigmoid
                    )
                    TY = pb.tile([128, 9, 5, R], BF16, tag="ty")
                    TX = pb.tile([128, 9, 5, R], BF16, tag="tx")
                    TA = pb1.tile([128, 9, R], BF16, tag="ta")
                    one = CSTB[:, 5:6]
                    for e in range(5):
                        nege = CSTB[:, 4 - e : 5 - e]
                        nc.scalar.activation(
                            TA[:], OMT[:, 0:9, i0 : i0 + R], ACTF.Abs, bias=nege
                        )
                        nc.scalar.activation(
                            TY[:, :, e, :], TA[:], ACTF.Relu, bias=one, scale=-1.0
                        )
                        nc.scalar.activation(
                            TA[:], OMT[:, 9:18, i0 : i0 + R], ACTF.Abs, bias=nege
                        )
                        nc.scalar.activation(
                            TX[:, :, e, :], TA[:], ACTF.Relu, bias=one, scale=-1.0
                        )
                    MSK2 = pb.tile([128, 9, R], BF16, tag="msk2")
                    nc.scalar.activation(
                        MSK2[:], MSK[:], ACTF.Copy, scale=2.0
                    )
                    TYM = pb.tile([128, 9, 5, R], BF16, tag="tym")
                    nc.vector.tensor_tensor(
                        out=TYM[:], in0=TY[:],
                        in1=MSK2[:, :, None, :].broadcast_to([128, 9, 5, R]),
                        op=ALU.mult,
                    )
                    # CT2[j, kx, ey, ex, ky, i] = (TYM*2) * TX
                    CT2 = pb1.tile([128, 3, 5, 5, 3, R], BF16, tag="ct2")
                    tymb = TYM[:]
                    txb = TX[:]
                    for kx in range(3):
                        for eyi in range(5):
                            nc.vector.tensor_tensor(
                                out=CT2[:, kx, eyi],
                                in0=rap(TYM, kx * 5 * R + eyi * R,
                                        [[9 * 5 * R, 128], [0, 5], [15 * R, 3],
                                         [1, R]]),
                                in1=rap(TX, kx * 5 * R,
                                        [[9 * 5 * R, 128], [R, 5], [15 * R, 3],
                                         [1, R]]),
                                op=ALU.mult,
                            )

                    _dbg_regs(nc, f"band{ib} after tents")
                    # -- MAC
                    ACCA = pacc.tile([128, 3, O, R], BF16, tag="acca")
                    ACCB = pacc.tile([128, 3, O, R], BF16, tag="accb")
                    TMPA = pacc.tile([128, 3, O, R], BF16, tag="tmpa")
                    TMPB = pacc.tile([128, 3, O, R], BF16, tag="tmpb")

                    def vap(kx, ey, ex):
                        sx = kx - 1 + ex
                        if abs(ex) <= 1:
                            return rap(
                                VT2, MAIN_IDX[(sx, kx)] * PBLK + (ey + 2 + S0),
                                [[9 * PBLK, 128], [KYSTR + 1, 3], [SLEN, O],
                                 [1, R]],
                            )
                        return rap(
                            VT3, OUT_IDX[(sx, kx)] * PBLK3 + (ey + 1),
                            [[6 * PBLK3, 128], [KYSTR3 + 1, 3], [SLEN3, O],
                             [1, R]],
                        )

                    def cap(kx, ey, ex):
                        off = ((kx * 5 + (ey + 2)) * 5 + (ex + 2)) * 3 * R
                        return rap(
                            CT2, off,
                            [[3 * 5 * 5 * 3 * R, 128], [R, 3], [0, O], [1, R]],
                        )

                    def term(eng, acc, tmp, kx, ey, ex, first):
                        if first:
                            eng.tensor_tensor(
                                out=acc[:], in0=vap(kx, ey, ex),
                                in1=cap(kx, ey, ex), op=ALU.mult,
                            )
                        else:
                            eng.tensor_tensor(
                                out=tmp[:], in0=vap(kx, ey, ex),
                                in1=cap(kx, ey, ex), op=ALU.mult,
                            )
                            eng.tensor_tensor(
                                out=acc[:], in0=acc[:], in1=tmp[:], op=ALU.add
                            )

                    # gpsimd: center terms (odd-parity on DVE anyway)
                    for gi, kx in enumerate(range(3)):
                        term(nc.gpsimd, ACCB, TMPB, kx, 0, 0, gi == 0)
                    # DVE inner (minus center)
                    di = 0
                    for kx in range(3):
                        for ey, ex in INNER:
                            if (ey, ex) == (0, 0):
                                continue
                            term(nc.vector, ACCA, TMPA, kx, ey, ex, di == 0)
                            di += 1
                    # DVE outer, dynamically skipped
                    if dyn_skip:
                        nc.vector.drain()
                        for kx in range(3):
                            rf = nc.vector.alloc_register(f"fy{ib}_{kx}")
                            nc.vector.reg_load(
                                rf, FLS[0:1, 0, kx : kx + 1].bitcast(mybir.dt.int32)
                            )
                            with tc.If(bass.RuntimeValue(rf) == 0) as cy:
                                pass
                            with cy.Else():
                                for ey, ex in OUTER_Y:
                                    term(nc.vector, ACCA, TMPA, kx, ey, ex, False)
                            nc.vector.free_register(rf)
                            rf = nc.vector.alloc_register(f"fx{ib}_{kx}")
                            nc.vector.reg_load(
                                rf, FLS[0:1, 1, kx : kx + 1].bitcast(mybir.dt.int32)
                            )
                            with tc.If(bass.RuntimeValue(rf) == 0) as cx:
                                pass
                            with cx.Else():
                                for ey, ex in OUTER_X:
                                    term(nc.vector, ACCA, TMPA, kx, ey, ex, False)
                            nc.vector.free_register(rf)
                    else:
                        for kx in range(3):
                            for ey, ex in OUTER_Y + OUTER_X:
                                term(nc.vector, ACCA, TMPA, kx, ey, ex, False)
                    # unconditional touch of VT3 (WAR sem for next band's writes)
                    DUM = pb1.tile([128, 1], BF16, tag="dum")
                    nc.vector.tensor_scalar(
                        out=DUM[:], in0=VT3[:, 0:1], scalar1=1.0, scalar2=None,
                        op0=ALU.mult,
                    )

                    # -- collapse + out
                    nc.vector.tensor_tensor(
                        out=ACCA[:], in0=ACCA[:], in1=ACCB[:], op=ALU.add
                    )
                    nc.vector.tensor_tensor(
                        out=ACCA[:, 0], in0=ACCA[:, 0], in1=ACCA[:, 1], op=ALU.add
                    )
                    nc.vector.tensor_tensor(
                        out=ACCA[:, 0], in0=ACCA[:, 0], in1=ACCA[:, 2], op=ALU.add
                    )
                    FIN = pb.tile([128, O, R], BF16, tag="fin")
                    nc.scalar.copy(FIN[:], ACCA[:, 0])
                    nc.sync.dma_start(
                        outD[:, ib], FIN[:].rearrange("p a b -> p (a b)")
                    )

    if fix_waits:
        _fix_multiwait(nc)
    return nc


def make_consts(w_off, b_off, w_mod, b_mod, w_reg):
    wconv = np.zeros((128, KK, 27), np.float32)
    for t in range(KK):
        ty, tx = t // 3, t % 3
        wconv[0:64, t, 0:18] = w_off[:, :, ty, tx].T     # z half -> offsets
        wconv[64:128, t, 18:27] = w_mod[:, :, ty, tx].T  # x half -> mask
    # reorder offset channels so planes are [dy*9, dx*9, mask*9]
    perm = list(range(0, 18, 2)) + list(range(1, 18, 2)) + list(range(18, 27))
    wconv = wconv[:, :, perm]
    wcz = wconv[0:64].reshape(64, KK * 27)
    wcx = wconv[64:128].reshape(64, KK * 27)
    w3 = w_reg.reshape(O, C, 3, 3)  # [o, c, ky, kx]
    wflat = np.ascontiguousarray(w3.transpose(1, 3, 2, 0).reshape(C, 3 * 3 * O))
    bias27 = np.concatenate([b_off[perm[:18]], b_mod]).reshape(27, 1).astype(
        np.float32
    )
    identb = np.eye(32, dtype=np.float32)
    cstb = np.tile(
        np.array([-2.0, -1.0, 0.0, 1.0, 2.0, 1.0, -0.99, 0.0], np.float32), (128, 1)
    )
    onesb = np.ones((128, 1), np.float32)
    return wcz, wcx, wflat, bias27, identb, cstb, onesb


def make_pad(img):
    p = np.zeros((64, PW, PW), np.float32)
    p[:, 3 : 3 + H, 3 : 3 + W] = img
    return p.reshape(64, PW * PW)


_NC_CACHE = None


def _get_nc():
    global _NC_CACHE
    if _NC_CACHE is None:
        _NC_CACHE = build_nc()
    return _NC_CACHE


def _make_in_maps(inp):
    import ml_dtypes

    bf = ml_dtypes.bfloat16
    x = np.asarray(inp["x"], np.float32)
    z = np.asarray(inp["z"], np.float32)
    wcz, wcx, wflat, bias27, identb, cstb, onesb = make_consts(
        np.asarray(inp["w_off"], np.float32), np.asarray(inp["b_off"], np.float32),
        np.asarray(inp["w_mod"], np.float32), np.asarray(inp["b_mod"], np.float32),
        np.asarray(inp["w_reg"], np.float32),
    )
    in_maps = []
    for b in range(x.shape[0]):
        in_maps.append(
            dict(
                zin=make_pad(z[b]).astype(bf),
                xin=make_pad(x[b]).astype(bf),
                wcz=wcz.astype(bf),
                wcx=wcx.astype(bf),
                wflat=wflat.astype(bf),
                bias27=bias27,
                identb=identb.astype(bf),
                cstb=cstb.astype(bf),
                onesb=onesb.astype(bf),
            )
        )
    return in_maps


def kernel(x, z, w_off, b_off, w_mod, b_mod, w_reg):
    in_maps = _make_in_maps(
        dict(x=x, z=z, w_off=w_off, b_off=b_off, w_mod=w_mod, b_mod=b_mod,
             w_reg=w_reg)
    )
    nc = _get_nc()
    res = run_bass_kernel_spmd(nc, in_maps, list(range(NCORES)))
    outs = []
    for b in range(len(in_maps)):
        arr = np.asarray(res.results[b]["outD"], np.float32).reshape(128, NBANDS, O, R)
        outs.append(
            np.ascontiguousarray(arr.transpose(2, 1, 3, 0)).reshape(O, H, W)
        )
    return np.stack(outs).astype(np.float32)

